# revision 1
# baseline (speedup 1.0000x reference)
"""CosineEmbeddingLoss (B=8192, D=128) on 8 TRN2 NeuronCores.

Data-parallel: each core gets a [1024,128] anchor slab + the full
[8192,128] positive matrix (bf16). Per core:
  - normalize positive rows (ttr sumsq -> rsqrt -> scale), DMA-xbar
    transpose to pT [128, 8192] bf16
  - transpose raw anchor slab to aT [128, 1024] bf16 (row scale folded
    in after the reduction: relu(c*x) = c*relu(x) for c>0)
  - 128 bf16 matmuls [K=128, M=128, N=512] -> PSUM [128,2048] groups
  - relu+row-sum of each group on ScalarE (activation Relu accum_out)
    or VectorE (tensor_tensor_reduce max/add), split for balance
  - diagonal correction from row-dots of matching anchor/positive rows
Host sums the 8 partial scalars, adds B (the +1 per diagonal term) and
divides by B*B.
"""

import numpy as np
import ml_dtypes

import concourse.bass as bass
import concourse.tile as tile
from concourse import bacc, mybir
from concourse.bass_utils import run_bass_kernel_spmd

B, D, NCORES = 8192, 128, 8
SLAB = B // NCORES          # 1024 anchor rows per core
PT = B // 128               # 64 positive tiles of 128 rows
AT = SLAB // 128            # 8 anchor tiles
NGRP = 4                    # [128, 2048] psum groups per m-block
GRPN = 2048
MMN = 512                   # matmul free dim
F32 = mybir.dt.float32
BF16 = mybir.dt.bfloat16

_CACHE: dict = {}


def _use_act(idx: int) -> bool:
    # ~18 of 32 groups on ScalarE (570ns/group) vs VectorE (658ns/group)
    return (idx * 9) // 16 != ((idx + 1) * 9) // 16


def _body(tc, a_in, p_in, pd_in, out):
    nc = tc.nc
    Relu = mybir.ActivationFunctionType.Relu
    Sqrt = mybir.ActivationFunctionType.Sqrt
    Square = mybir.ActivationFunctionType.Square
    mult, add, amax = mybir.AluOpType.mult, mybir.AluOpType.add, mybir.AluOpType.max
    sub = mybir.AluOpType.subtract
    X = mybir.AxisListType.X

    import contextlib
    ctx = contextlib.ExitStack()
    with ctx:
        singles = ctx.enter_context(tc.tile_pool(name="singles", bufs=1))
        ptiles = ctx.enter_context(tc.tile_pool(name="ptiles", bufs=6))
        phat = ctx.enter_context(tc.tile_pool(name="phat", bufs=6))
        junkp = ctx.enter_context(tc.tile_pool(name="junkp", bufs=3))
        prep_ctx = contextlib.ExitStack()
        tpsum = prep_ctx.enter_context(
            tc.tile_pool(name="tpsum", bufs=4, space="PSUM"))

        # persistent buffers
        pT = singles.tile([128, B], BF16)            # transposed normalized positive
        aT = singles.tile([128, SLAB], BF16)         # transposed raw anchor
        sumsq_p = singles.tile([128, PT], F32)
        rsq_p = singles.tile([128, PT], F32)
        sumsq_a = singles.tile([128, AT], F32)
        rsq_a = singles.tile([128, AT], F32)
        sumsq_pd = singles.tile([128, AT], F32)
        rsq_pd = singles.tile([128, AT], F32)
        draw = singles.tile([128, AT], F32)          # raw diag dots
        racc_a = singles.tile([128, 32], F32)        # ScalarE group sums
        racc_d = singles.tile([128, 32], F32)        # VectorE group sums
        zeros = singles.tile([128, GRPN], BF16)
        dummy = singles.tile([128, 1], F32)
        sqscr = singles.tile([128, D], BF16)
        sqf32 = singles.tile([128, D], F32)
        from concourse.masks import make_identity
        ident = singles.tile([128, 128], BF16)
        make_identity(nc, ident[:])
        nc.vector.memset(racc_a[:], 0.0)
        nc.vector.memset(racc_d[:], 0.0)
        nc.vector.memset(zeros[:], 0.0)

        p_r = p_in.rearrange("(n p) d -> n p d", p=128)
        a_r = a_in.rearrange("(n p) d -> n p d", p=128)
        pd_r = pd_in.rearrange("(n p) d -> n p d", p=128)

        # ---- positive: load+sumsq per 16-tile batch, rsqrt, scale+transpose ----
        p_nat = singles.tile([128, B], BF16)
        for q in range(PT // 16):
            for t in range(q * 16, (q + 1) * 16):
                pn = p_nat[:, t * 128 : (t + 1) * 128]
                nc.sync.dma_start(out=pn, in_=p_r[t])
                nc.scalar.activation(
                    out=sqscr[:], in_=pn, func=Square,
                    accum_out=sumsq_p[:, t : t + 1])
            sl = slice(q * 16, (q + 1) * 16)
            nc.scalar.activation(
                out=rsq_p[:, sl], in_=sumsq_p[:, sl], func=Sqrt)
            nc.vector.reciprocal(out=rsq_p[:, sl], in_=rsq_p[:, sl])
            for t in range(q * 16, (q + 1) * 16):
                ph = phat.tile([128, D], BF16, tag="ph")
                nc.vector.tensor_scalar(
                    out=ph[:], in0=p_nat[:, t * 128 : (t + 1) * 128],
                    scalar1=rsq_p[:, t : t + 1], scalar2=None, op0=mult)
                tp = tpsum.tile([128, 128], BF16, tag="tp")
                nc.tensor.transpose(tp[:], ph[:], ident[:])
                nc.vector.tensor_copy(
                    out=pT[:, t * 128 : (t + 1) * 128], in_=tp[:])

        # ---- anchor: load, sumsq, transpose raw ----
        for t in range(AT):
            at = ptiles.tile([128, D], BF16, tag="at")
            nc.sync.dma_start(out=at[:], in_=a_r[t])
            nc.scalar.activation(
                out=sqscr[:], in_=at[:], func=Square,
                accum_out=sumsq_a[:, t : t + 1])
            tp = tpsum.tile([128, 128], BF16, tag="tp")
            nc.tensor.transpose(tp[:], at[:], ident[:])
            nc.vector.tensor_copy(
                out=aT[:, t * 128 : (t + 1) * 128], in_=tp[:])
            # matching positive rows for the diagonal
            pdt = ptiles.tile([128, D], BF16, tag="pdt")
            nc.sync.dma_start(out=pdt[:], in_=pd_r[t])
            nc.scalar.activation(
                out=sqscr[:], in_=pdt[:], func=Square,
                accum_out=sumsq_pd[:, t : t + 1])
            nc.vector.tensor_tensor(out=sqf32[:], in0=at[:], in1=pdt[:], op=mult)
            nc.vector.tensor_reduce(
                out=draw[:, t : t + 1], in_=sqf32[:], axis=X, op=add)
        nc.scalar.activation(out=rsq_a[:], in_=sumsq_a[:], func=Sqrt)
        nc.vector.reciprocal(out=rsq_a[:], in_=rsq_a[:])
        nc.scalar.activation(out=rsq_pd[:], in_=sumsq_pd[:], func=Sqrt)
        nc.vector.reciprocal(out=rsq_pd[:], in_=rsq_pd[:])

        prep_ctx.close()
        psum = ctx.enter_context(tc.tile_pool(name="psum", bufs=2, space="PSUM"))

        # ---- main loop ----
        for g in range(NGRP):
            for m in range(AT):
                ps = psum.tile([128, GRPN], F32, tag="ps")
                for j in range(GRPN // MMN):
                    col = g * GRPN + j * MMN
                    nc.tensor.matmul(
                        out=ps[:, j * MMN : (j + 1) * MMN],
                        lhsT=aT[:, m * 128 : (m + 1) * 128],
                        rhs=pT[:, col : col + MMN],
                        start=True, stop=True)
                idx = g * AT + m
                junk = junkp.tile([128, GRPN], BF16, tag="junk")
                if idx % 3 != 0:
                    nc.scalar.activation(
                        out=junk[:], in_=ps[:], func=Relu,
                        accum_out=racc_a[:, idx : idx + 1])
                else:
                    nc.vector.tensor_scalar(
                        out=junk[:], in0=ps[:], scalar1=0.0, scalar2=None,
                        op0=amax)
                    nc.vector.tensor_reduce(
                        out=racc_d[:, idx : idx + 1], in_=junk[:], axis=X,
                        op=add)

        # ---- combine ----
        racc_s = singles.tile([128, 32], F32)
        nc.vector.tensor_add(racc_s[:], racc_a[:], racc_d[:])
        rowsum = singles.tile([128, AT], F32)
        racc3 = racc_s.rearrange("p (g m) -> p g m", g=NGRP)
        nc.vector.tensor_add(rowsum[:], racc3[:, 0, :], racc3[:, 1, :])
        nc.vector.tensor_add(rowsum[:], rowsum[:], racc3[:, 2, :])
        nc.vector.tensor_add(rowsum[:], rowsum[:], racc3[:, 3, :])
        # scale relu-sums by r_a; diag cos = draw * r_a * r_pd
        nc.vector.tensor_mul(rowsum[:], rowsum[:], rsq_a[:])
        dcos = singles.tile([128, AT], F32)
        nc.vector.tensor_mul(dcos[:], draw[:], rsq_a[:])
        nc.vector.tensor_mul(dcos[:], dcos[:], rsq_pd[:])
        drelu = singles.tile([128, AT], F32)
        nc.scalar.activation(out=drelu[:], in_=dcos[:], func=Relu)
        # contrib = rowsum - dcos - drelu   (the +1 per diag added on host)
        nc.vector.tensor_tensor(rowsum[:], rowsum[:], dcos[:], op=sub)
        nc.vector.tensor_tensor(rowsum[:], rowsum[:], drelu[:], op=sub)
        total = singles.tile([128, 1], F32)
        nc.vector.tensor_reduce(total[:], rowsum[:], axis=X, op=add)
        from concourse.bass_isa import ReduceOp
        nc.gpsimd.partition_all_reduce(total[:], total[:], 128, ReduceOp.add)
        nc.sync.dma_start(out=out[:], in_=total[0:1, 0:1])


def _build():
    nc = bacc.Bacc("TRN2", target_bir_lowering=False, debug=False,
                   num_devices=NCORES)
    a_in = nc.declare_dram_parameter("a", [SLAB, D], BF16, isOutput=False)
    p_in = nc.declare_dram_parameter("p", [B, D], BF16, isOutput=False)
    pd_in = nc.declare_dram_parameter("pd", [SLAB, D], BF16, isOutput=False)
    out = nc.declare_dram_parameter("out", [1, 1], F32, isOutput=True)
    with tile.TileContext(nc) as tc:
        _body(tc, a_in[:], p_in[:], pd_in[:], out[:])
    nc.compile()
    return nc


def kernel(hid_positive: np.ndarray, hid_anchor: np.ndarray, **run_kwargs):
    if "nc" not in _CACHE:
        _CACHE["nc"] = _build()
    nc = _CACHE["nc"]
    p16 = np.asarray(hid_positive, dtype=np.float32).astype(ml_dtypes.bfloat16)
    a16 = np.asarray(hid_anchor, dtype=np.float32).astype(ml_dtypes.bfloat16)
    in_maps = []
    for c in range(NCORES):
        sl = slice(c * SLAB, (c + 1) * SLAB)
        in_maps.append({"a": a16[sl], "p": p16, "pd": p16[sl]})
    res = run_bass_kernel_spmd(nc, in_maps, core_ids=list(range(NCORES)),
                               **run_kwargs)
    s = sum(float(res.results[c]["out"][0, 0]) for c in range(NCORES))
    loss = np.float32((s + B) / (float(B) * float(B)))
    if run_kwargs:
        _CACHE["last_result"] = res
    return np.asarray(loss, dtype=np.float32)



# revision 18
# speedup vs baseline: 1.0785x; 1.0785x over previous
"""CosineEmbeddingLoss (B=8192, D=128) on 8 TRN2 NeuronCores.

Data-parallel over anchor rows: each core gets a [1024,128] anchor slab
plus the full [8192,128] positive matrix (bf16). Per core:
  - positives: chunked pipeline (8 chunks of 1024 rows): natural-layout
    load -> sumsq (Square + grouped 3D reduce) -> rsqrt -> per-row scale
    (per-partition tensor_scalar) -> store scaled rows to DRAM scratch ->
    xbar transpose-DMA back as p^T [128, 8192] bf16
  - anchors: raw transpose-DMA straight from DRAM (row norms folded in
    after the reduction: relu(c*x) = c*relu(x) for c>0)
  - main loop: 128 bf16 matmuls [K=128, M=128, N=512] -> PSUM [128,2048]
    groups; each group consumed in ONE pass by either ScalarE
    (activation Relu + accum_out) or VectorE (tensor_tensor_reduce
    max/add vs zeros)
  - diagonal correction from row-dots of matching anchor/positive rows
  - final 128-partition reduction via a tiny fp32 ones-matmul
Host sums the 8 partial scalars, adds B (the +1 per diagonal term) and
divides by B*B.
"""

import contextlib

import numpy as np
import ml_dtypes

import concourse.bass as bass
import concourse.tile as tile
from concourse import bacc, mybir
from concourse.bass_utils import run_bass_kernel_spmd

B, D, NCORES = 8192, 128, 8
SLAB = B // NCORES          # 1024 anchor rows per core
AT = SLAB // 128            # 8 anchor tiles
NCHUNK = 8                  # positive chunks
CH = B // NCHUNK            # 1024 positive rows per chunk
CHT = CH // 128             # 8 tiles per chunk
NJG = 4                     # j-groups of 2048 columns
GRPN = 2048
MMN = 512                   # matmul free dim
NGRP = NJG * AT             # 32 psum groups
S_GROUPS = 17               # groups consumed on ScalarE (rest VectorE)
F32 = mybir.dt.float32
BF16 = mybir.dt.bfloat16

import os
TAIL_MM = os.environ.get("K_TAIL_MM", "1") == "1"      # ones-matmul tail vs gpsimd
AT_DMA = os.environ.get("K_AT_DMA", "1") == "1"        # aT via transpose-DMA vs PE
ROUNDTRIP = os.environ.get("K_ROUNDTRIP", "1") == "1"  # p^T via DRAM roundtrip vs PE

_CACHE: dict = {}


def _use_scalar(idx: int) -> bool:
    return (idx * S_GROUPS) // NGRP != ((idx + 1) * S_GROUPS) // NGRP


def _body(tc, a_in, p_in, pd_in, out):
    nc = tc.nc
    Relu = mybir.ActivationFunctionType.Relu
    Sqrt = mybir.ActivationFunctionType.Sqrt
    Square = mybir.ActivationFunctionType.Square
    mult, add, amax = mybir.AluOpType.mult, mybir.AluOpType.add, mybir.AluOpType.max
    sub = mybir.AluOpType.subtract
    X = mybir.AxisListType.X

    ctx = contextlib.ExitStack()
    with ctx:
        singles = ctx.enter_context(tc.tile_pool(name="singles", bufs=1))
        chunks = ctx.enter_context(tc.tile_pool(name="chunks", bufs=2))
        junkp = ctx.enter_context(tc.tile_pool(name="junkp", bufs=3))
        dramp = ctx.enter_context(tc.tile_pool(name="dramp", bufs=1, space="DRAM"))
        main_ctx = contextlib.ExitStack()

        # persistent tiles (p^T as separate per-chunk tiles: transpose-DMA
        # needs a contiguous SBUF destination)
        pTc = [singles.tile([128, CH], BF16, name=f"pTc{k}")
               for k in range(NCHUNK)]
        aT = singles.tile([128, SLAB], BF16)       # transposed raw anchors
        a_nat = singles.tile([128, SLAB], BF16)
        pd_nat = singles.tile([128, SLAB], BF16)
        sumsq_p = singles.tile([128, NCHUNK * CHT], F32)
        rsq_p = singles.tile([128, NCHUNK * CHT], F32)
        sumsq_a = singles.tile([128, AT], F32)
        rsq_a = singles.tile([128, AT], F32)
        sumsq_pd = singles.tile([128, AT], F32)
        rsq_pd = singles.tile([128, AT], F32)
        draw = singles.tile([128, AT], F32)
        dcorr = singles.tile([128, AT], F32)
        racc_s = singles.tile([128, NGRP], F32)    # ScalarE group sums
        racc_v = singles.tile([128, NGRP], F32)    # VectorE group sums
        zeros = singles.tile([128, GRPN], BF16)
        dj = singles.tile([128, SLAB], BF16)       # diag product scratch
        ones1 = singles.tile([128, 1], F32)
        tot = singles.tile([128, 1], F32)
        phat_dram = dramp.tile([B, D], BF16)

        nc.vector.memset(racc_s[:], 0.0)
        nc.vector.memset(racc_v[:], 0.0)
        nc.vector.memset(zeros[:], 0.0)
        nc.vector.memset(ones1[:], 1.0)
        if not (AT_DMA and ROUNDTRIP):
            from concourse.masks import make_identity
            ident = singles.tile([128, 128], BF16)
            make_identity(nc, ident[:])
            tpsum_ctx = contextlib.ExitStack()
            tpsum = tpsum_ctx.enter_context(
                tc.tile_pool(name="tpsum", bufs=2, space="PSUM"))

        # natural-layout DRAM views: row (c*CH + t*128 + p) -> [c][p, t, d]
        p_r = p_in.rearrange("(c t p) d -> c p t d", p=128, t=CHT)
        a_r = a_in.rearrange("(t p) d -> p t d", p=128)
        pd_r = pd_in.rearrange("(t p) d -> p t d", p=128)
        pd_w = phat_dram[:].rearrange("(c t p) d -> c p t d", p=128, t=CHT)

        # ---- input DMAs (emit first so they start early) ----
        nc.sync.dma_start(out=a_nat.rearrange("p (t d) -> p t d", d=D), in_=a_r)
        nc.sync.dma_start(out=pd_nat.rearrange("p (t d) -> p t d", d=D), in_=pd_r)
        if AT_DMA:
            nc.sync.dma_start_transpose(aT[:], a_in)
        else:
            for t in range(AT):
                tp = tpsum.tile([128, 128], BF16, tag="tp")
                nc.tensor.transpose(
                    tp[:], a_nat[:, t * 128 : (t + 1) * 128], ident[:])
                nc.vector.tensor_copy(
                    out=aT[:, t * 128 : (t + 1) * 128], in_=tp[:])

        # ---- positive chunks: load -> sumsq -> rsqrt -> scale -> roundtrip ----
        for k in range(NCHUNK):
            pn = chunks.tile([128, CH], BF16, tag="pn")
            nc.sync.dma_start(
                out=pn.rearrange("p (t d) -> p t d", d=D), in_=p_r[k])
            csl = slice(k * CHT, (k + 1) * CHT)
            sqj = chunks.tile([128, CH], BF16, tag="sqj")
            if k % 2 == 0:
                nc.scalar.activation(out=sqj[:], in_=pn[:], func=Square)
            else:
                nc.vector.tensor_tensor(out=sqj[:], in0=pn[:], in1=pn[:], op=mult)
            nc.vector.tensor_reduce(
                out=sumsq_p[:, csl],
                in_=sqj.rearrange("p (t d) -> p t d", d=D),
                axis=X, op=add)
            nc.scalar.activation(
                out=rsq_p[:, csl], in_=sumsq_p[:, csl], func=Sqrt)
            nc.vector.reciprocal(out=rsq_p[:, csl], in_=rsq_p[:, csl])
            ph = chunks.tile([128, CH], BF16, tag="ph")
            for t in range(CHT):
                nc.vector.tensor_scalar(
                    out=ph[:, t * D : (t + 1) * D],
                    in0=pn[:, t * D : (t + 1) * D],
                    scalar1=rsq_p[:, k * CHT + t : k * CHT + t + 1],
                    scalar2=None, op0=mult)
            if ROUNDTRIP:
                nc.sync.dma_start(
                    out=pd_w[k], in_=ph.rearrange("p (t d) -> p t d", d=D))
                nc.sync.dma_start_transpose(
                    out=pTc[k][:],
                    in_=phat_dram[k * CH : (k + 1) * CH, :])
            else:
                for t in range(CHT):
                    tp = tpsum.tile([128, 128], BF16, tag="tp")
                    nc.tensor.transpose(
                        tp[:], ph[:, t * D : (t + 1) * D], ident[:])
                    nc.vector.tensor_copy(
                        out=pTc[k][:, t * 128 : (t + 1) * 128], in_=tp[:])

        # ---- anchor + diag prep ----
        sqa = chunks.tile([128, CH], BF16, tag="sqj")
        nc.scalar.activation(out=sqa[:], in_=a_nat[:], func=Square)
        nc.vector.tensor_reduce(
            out=sumsq_a[:], in_=sqa.rearrange("p (t d) -> p t d", d=D),
            axis=X, op=add)
        sqpd = chunks.tile([128, CH], BF16, tag="sqj")
        nc.scalar.activation(out=sqpd[:], in_=pd_nat[:], func=Square)
        nc.vector.tensor_reduce(
            out=sumsq_pd[:], in_=sqpd.rearrange("p (t d) -> p t d", d=D),
            axis=X, op=add)
        nc.vector.tensor_tensor(out=dj[:], in0=a_nat[:], in1=pd_nat[:], op=mult)
        nc.vector.tensor_reduce(
            out=draw[:], in_=dj.rearrange("p (t d) -> p t d", d=D),
            axis=X, op=add)
        nc.scalar.activation(out=rsq_a[:], in_=sumsq_a[:], func=Sqrt)
        nc.vector.reciprocal(out=rsq_a[:], in_=rsq_a[:])
        nc.scalar.activation(out=rsq_pd[:], in_=sumsq_pd[:], func=Sqrt)
        nc.vector.reciprocal(out=rsq_pd[:], in_=rsq_pd[:])
        # dcos = draw * rsq_a * rsq_pd ; dcorr = dcos + relu(dcos)
        nc.vector.tensor_mul(draw[:], draw[:], rsq_a[:])
        nc.vector.tensor_mul(draw[:], draw[:], rsq_pd[:])
        nc.scalar.activation(out=dcorr[:], in_=draw[:], func=Relu)
        nc.vector.tensor_add(dcorr[:], dcorr[:], draw[:])

        # ---- main loop: 4 j-groups x 8 m-tiles ----
        if not (AT_DMA and ROUNDTRIP):
            tpsum_ctx.close()
        psum = main_ctx.enter_context(
            tc.tile_pool(name="psum", bufs=2, space="PSUM"))
        for jg in range(NJG):
            for m in range(AT):
                ps = psum.tile([128, GRPN], F32, tag="ps")
                for u in range(GRPN // MMN):
                    ck = jg * 2 + u // 2
                    off = (u % 2) * MMN
                    nc.tensor.matmul(
                        out=ps[:, u * MMN : (u + 1) * MMN],
                        lhsT=aT[:, m * 128 : (m + 1) * 128],
                        rhs=pTc[ck][:, off : off + MMN],
                        start=True, stop=True)
                idx = m * NJG + jg
                junk = junkp.tile([128, GRPN], BF16, tag="junk")
                if _use_scalar(idx):
                    nc.scalar.activation(
                        out=junk[:], in_=ps[:], func=Relu,
                        accum_out=racc_s[:, idx : idx + 1])
                else:
                    nc.vector.tensor_scalar(
                        out=junk[:], in0=ps[:], scalar1=0.0, scalar2=0.0,
                        op0=amax, op1=add,
                        accum_out=racc_v[:, idx : idx + 1])

        # ---- combine ----
        nc.vector.tensor_add(racc_s[:], racc_s[:], racc_v[:])
        rowsum = singles.tile([128, AT], F32)
        nc.vector.tensor_reduce(
            out=rowsum[:],
            in_=racc_s.rearrange("p (m g) -> p m g", g=NJG),
            axis=X, op=add)
        nc.vector.tensor_mul(rowsum[:], rowsum[:], rsq_a[:])
        nc.vector.tensor_tensor(rowsum[:], rowsum[:], dcorr[:], op=sub)
        nc.vector.tensor_reduce(out=tot[:], in_=rowsum[:], axis=X, op=add)

        main_ctx.close()
        if TAIL_MM:
            tail = ctx.enter_context(
                tc.tile_pool(name="tail", bufs=1, space="PSUM"))
            ps1 = tail.tile([1, 1], F32)
            nc.tensor.matmul(out=ps1[:], lhsT=tot[:], rhs=ones1[:],
                             start=True, stop=True)
            res = singles.tile([1, 1], F32)
            nc.vector.tensor_copy(out=res[:], in_=ps1[:])
            nc.sync.dma_start(out=out[:], in_=res[:])
        else:
            from concourse.bass_isa import ReduceOp
            nc.gpsimd.partition_all_reduce(tot[:], tot[:], 128, ReduceOp.add)
            nc.sync.dma_start(out=out[:], in_=tot[0:1, 0:1])


def _build():
    nc = bacc.Bacc("TRN2", target_bir_lowering=False, debug=False,
                   num_devices=NCORES)
    a_in = nc.declare_dram_parameter("a", [SLAB, D], BF16, isOutput=False)
    p_in = nc.declare_dram_parameter("p", [B, D], BF16, isOutput=False)
    pd_in = nc.declare_dram_parameter("pd", [SLAB, D], BF16, isOutput=False)
    out = nc.declare_dram_parameter("out", [1, 1], F32, isOutput=True)
    with tile.TileContext(nc) as tc:
        _body(tc, a_in[:], p_in[:], pd_in[:], out[:])
    nc.compile()
    return nc


def kernel(hid_positive: np.ndarray, hid_anchor: np.ndarray, **run_kwargs):
    if "nc" not in _CACHE:
        _CACHE["nc"] = _build()
    nc = _CACHE["nc"]
    p16 = np.asarray(hid_positive, dtype=np.float32).astype(ml_dtypes.bfloat16)
    a16 = np.asarray(hid_anchor, dtype=np.float32).astype(ml_dtypes.bfloat16)
    in_maps = []
    for c in range(NCORES):
        sl = slice(c * SLAB, (c + 1) * SLAB)
        in_maps.append({"a": a16[sl], "p": p16, "pd": p16[sl]})
    res = run_bass_kernel_spmd(nc, in_maps, core_ids=list(range(NCORES)),
                               **run_kwargs)
    s = sum(float(res.results[c]["out"][0, 0]) for c in range(NCORES))
    loss = np.float32((s + B) / (float(B) * float(B)))
    if run_kwargs:
        _CACHE["last_result"] = res
    return np.asarray(loss, dtype=np.float32)
